# revision 34
# baseline (speedup 1.0000x reference)
"""Trainium2 Bass kernel v4 for nn_DPP: batched masked-Gram logdet minus
shared normalizer logdet.

out[i] = logdet(G * m_i m_i^T + diag(1-m_i)) - logdet(G + I),  G = B^T B

Sharding: one sample per NeuronCore (B replicated); each core computes its
masked logdet AND the shared logdet; host gathers 8 scalars.

v4 changes vs v3 (~340us -> target <220us): the v3 bottleneck was the
per-round pivot-refine serial chain (~15us x 16 rounds of cross-engine
latency).  v4:
  - TRSM-ready factor is W = D^{-1/2}(I - X1 + X1@X1 + striuh(X1^T X1))
    (2nd-order inverse-Cholesky incl. the A2 term); the critical chain
    ends at wfac.  logdet corrections  tr(F) = sum(W o SW) - P  and
    -tr(F^2)/2  (F = W^T S W - I) run OFF the critical path, as does the
    exact base term -2 sum ln diag(W) (diag extracted from wfac).
  - Diagonal extracted as a ROW via GpSimd partition-reduce so the rsqrt
    row feeds the outer products without any PE transpose; a parallel
    column-form rsqrt feeds the W row-scaling.
  - Masked+shared refines share [P, 2, 128] tiles: one elementwise op
    per step covers both matrices on dual rounds.
  - U panels stored as fp8 pairs (u8 = +U, n8 = -U); accumulations are
    fp8 DoubleRow matmuls over TWO panels at once.  Odd tails use
    single-slot fp8 matmuls (no zero-init needed).
  - All fp8 DR matmuls use 512-col chunks (1024-elem moving AP).
  - TRSM diag-block chunk eliminated (its output is read by nothing).
  - Pipeline: after a panel's first TRSM chunk evacuates, the next
    pivot's single accum-tail matmul fires and the next refine's row
    chain starts under the remaining TRSM + chains.
"""

import numpy as np
import ml_dtypes

P = 128
N = 2048
NM = 1152
NT = N // P         # 16 shared panels
MT = NM // P        # 9 masked panels
NKT = 16            # contraction tiles (2000 rows padded to 2048)
FT = 512
DRT = 512
DW = 512
SCALE = 32.0
S2 = SCALE * SCALE
LN_S2 = float(np.log(S2))
# scale fix + per-refine -1.5P series constant (masked MT refines minus
# shared NT refines)
OUT_CONST = (N - NM) * LN_S2 + 1.5 * P * (NT - MT)

RM = [1, 3, 5, 7, 9, 11, 13, 14, 15]  # masked panel i -> round RM[i]

_CACHE = {}


def _chunks(width, base, step):
    out = []
    c = base
    end = base + width
    while c < end:
        w = min(step, end - c)
        out.append((c, w))
        c += w
    return out


def _build():
    import os
    import concourse.bass as bass
    import concourse.bacc as bacc
    import concourse.mybir as mybir
    from concourse.bass import ds
    from concourse.masks import (
        make_identity,
        make_upper_triangular,
        make_lower_triangular,
    )
    from concourse.tile import TileContext
    from contextlib import ExitStack

    f32 = mybir.dt.float32
    bf16 = mybir.dt.bfloat16
    fp8 = mybir.dt.float8e4
    AF = mybir.ActivationFunctionType
    OP = mybir.AluOpType
    DR = mybir.MatmulPerfMode.DoubleRow
    PSUM = bass.MemorySpace.PSUM
    AX = mybir.AxisListType.X
    AXY = mybir.AxisListType.XY
    AC = mybir.AxisListType.C

    nc = bacc.Bacc()
    bb = nc.dram_tensor("bb", [N, N], fp8, kind="ExternalInput")
    bs_d = nc.dram_tensor("bs", [N, NM], fp8, kind="ExternalInput")
    pad_d = nc.dram_tensor("pad", [NM, 1], f32, kind="ExternalInput")
    out_d = nc.dram_tensor("out", [1, 1], f32, kind="ExternalOutput")

    TDIM = [MT, NT]
    DIMW = [NM, N]
    NPAIR = [(MT + 1) // 2, (NT + 1) // 2]
    RESTW = [NM - DW, N - DW]     # rest-chain widths
    BANKW = [1024, 1536]          # rest psum capacity (banks)

    with TileContext(nc) as tc, ExitStack() as stack:
        consts = stack.enter_context(tc.tile_pool(name="consts", bufs=1))
        I128 = consts.tile([P, P], f32, tag="i128")
        make_identity(nc, I128)
        I2f = consts.tile([P, 2, P], f32, tag="i2f")
        nc.vector.tensor_copy(I2f[:, 0, :], I128)
        nc.vector.tensor_copy(I2f[:, 1, :], I128)
        STRIU = consts.tile([P, 2, P], f32, tag="striu")
        make_upper_triangular(nc, STRIU[:, 0, :], val=1.0, diag=False)
        nc.vector.tensor_copy(STRIU[:, 1, :], STRIU[:, 0, :])
        STRIL = consts.tile([P, 2, P], f32, tag="stril")
        make_lower_triangular(nc, STRIL[:, 0, :], val=1.0, diag=False)
        nc.vector.tensor_copy(STRIL[:, 1, :], STRIL[:, 0, :])
        STRIUH = consts.tile([P, 2, P], f32, tag="striuh")
        nc.vector.tensor_scalar(
            out=STRIUH[:, 0, :], in0=I128, scalar1=0.5, scalar2=None,
            op0=OP.mult,
        )
        nc.vector.tensor_add(STRIUH[:, 0, :], STRIUH[:, 0, :], STRIU[:, 0, :])
        nc.vector.tensor_copy(STRIUH[:, 1, :], STRIUH[:, 0, :])

        padc = consts.tile([P, MT], f32, tag="padc")
        nc.sync.dma_start(padc, pad_d.rearrange("(t p) one -> p (t one)", p=P))

        accA = consts.tile([P, 2], f32, tag="accA")
        nc.vector.memset(accA, 0.0)
        accB = consts.tile([P, 2], f32, tag="accB")
        nc.vector.memset(accB, 0.0)
        dstore = consts.tile([P, NT, 2], f32, tag="dstore")
        nc.vector.memset(dstore, 1.0)

        dfix_m = consts.tile([P, MT, P], f32, tag="dfix_m")
        pscl = consts.tile([P, MT], f32, tag="pscl")
        nc.vector.tensor_scalar(
            out=pscl, in0=padc, scalar1=S2, scalar2=None, op0=OP.mult
        )
        for i in range(MT):
            nc.vector.tensor_scalar_mul(dfix_m[:, i, :], I128, pscl[:, ds(i, 1)])
        dfix_s = consts.tile([P, P], f32, tag="dfix_s")
        nc.vector.tensor_scalar(
            out=dfix_s, in0=I128, scalar1=S2, scalar2=None, op0=OP.mult
        )

        bsel = consts.tile([P, NKT, NM], fp8, tag="bsel")
        bful = consts.tile([P, NKT, N], fp8, tag="bful")
        bs_r = bs_d.rearrange("(t p) n -> p t n", p=P)
        bb_r = bb.rearrange("(t p) n -> p t n", p=P)

        FP8ACC = os.environ.get("KV4_NOFP8", "0") != "1"
        SAFE = os.environ.get("KV4_SAFE", "0") == "1"
        SAFE_TTR = True  # tensor_tensor_reduce faults on this HW toolchain
        SAFE_STT = SAFE or os.environ.get("KV4_SAFE_STT", "0") == "1"
        SAFE_GSUB = SAFE or os.environ.get("KV4_SAFE_GSUB", "0") == "1"
        NROUNDS = int(os.environ.get("KV4_ROUNDS", str(NT)))
        edt = fp8 if FP8ACC else bf16
        # pair tile p only holds cols [2pP, DIM) (earlier cols never read)
        u8 = {}
        n8 = {}
        for m in range(2):
            for pr in range(NPAIR[m]):
                wp = DIMW[m] - 2 * pr * P
                u8[(m, pr)] = consts.tile(
                    [P, 2, wp], edt, tag=f"u8_{m}_{pr}", name=f"u8_{m}_{pr}"
                )
                n8[(m, pr)] = consts.tile(
                    [P, 2, wp], edt, tag=f"n8_{m}_{pr}", name=f"n8_{m}_{pr}"
                )

        ddiagp = stack.enter_context(tc.tile_pool(name="ddiag", bufs=1, space=PSUM))
        mrestp = stack.enter_context(tc.tile_pool(name="mrest", bufs=1, space=PSUM))
        srestp = stack.enter_context(tc.tile_pool(name="srest", bufs=1, space=PSUM))
        wpsump = stack.enter_context(tc.tile_pool(name="wpsum", bufs=1, space=PSUM))
        spool = stack.enter_context(tc.tile_pool(name="spool", bufs=2))
        rpool = stack.enter_context(tc.tile_pool(name="rpool", bufs=2))
        vpool = stack.enter_context(tc.tile_pool(name="vpool", bufs=2))

        ddiag = ddiagp.tile([P, 2, DW], f32, tag="ddiag", name="ddiag")
        wpsum = wpsump.tile([P, FT], f32, tag="wpsum", name="wpsum")
        RPOOL = [mrestp, srestp]

        X = [bsel, bful]

        def new_panel(m, i):
            T = TDIM[m]
            w = (T - i) * P
            cx = {"w": w, "dw": min(DW, w), "dp": ddiag[:, m, :],
                  "rp": None, "strip": None}
            if w > DW:
                cx["rp"] = RPOOL[m].tile([P, BANKW[m]], f32, tag=f"rp{m}",
                                         name="rp")
            if w > P:
                cx["strip"] = spool.tile([P, w - P], bf16, tag=f"strip{m}",
                                         name="strip")
            return cx

        def dr_gram(dst, m, kp, c0, cc, cw):
            nc.tensor.matmul(
                dst,
                X[m][:, 2 * kp : 2 * kp + 2, ds(c0, P)],
                X[m][:, 2 * kp : 2 * kp + 2, ds(cc, cw)],
                start=(kp == 0), stop=False, perf_mode=DR,
                skip_group_check=True,
            )

        def dr_acc(dst, m, pr, i, cc, cw, stop):
            o = 2 * pr * P
            if not FP8ACC:
                nc.tensor.matmul(
                    dst, n8[(m, pr)][:, 0, ds(i * P - o, P)],
                    u8[(m, pr)][:, 0, ds(cc - o, cw)],
                    start=False, stop=False, skip_group_check=True,
                )
                nc.tensor.matmul(
                    dst, n8[(m, pr)][:, 1, ds(i * P - o, P)],
                    u8[(m, pr)][:, 1, ds(cc - o, cw)],
                    start=False, stop=stop, skip_group_check=True,
                )
                return
            nc.tensor.matmul(
                dst,
                n8[(m, pr)][:, :, ds(i * P - o, P)],
                u8[(m, pr)][:, :, ds(cc - o, cw)],
                start=False, stop=stop, perf_mode=DR, skip_group_check=True,
            )

        def single_acc(dst, m, j, i, cc, cw, stop):
            pr, sl = j // 2, j % 2
            o = 2 * pr * P
            nc.tensor.matmul(
                dst,
                n8[(m, pr)][:, sl, ds(i * P - o, P)],
                u8[(m, pr)][:, sl, ds(cc - o, cw)],
                start=False, stop=stop, skip_group_check=True,
            )

        def diag_chain(m, i, cx, dma=None):
            """Gram + queue-time-available accums (panels <= i-2) for cols
            [iP, iP+dw).  The last term arrives via diag_tail."""
            dp, dw = cx["dp"], cx["dw"]
            c0 = i * P
            for kp in range(NKT // 2):
                if dma is not None:
                    dma(kp)
                dr_gram(dp[:, :dw], m, kp, c0, c0, dw)
                yield
            npr = (i - 1) // 2 if i % 2 == 1 else max(i - 2, 0) // 2
            for pr in range(npr):
                dr_acc(dp[:, :dw], m, pr, i, c0, dw, False)
                yield

        def diag_tail(m, i, cx, part):
            """part 0: pivot block [iP, iP+P) (stage-0 unblock);
            part 1: rest of diag region [iP+P, iP+dw)."""
            dp, dw = cx["dp"], cx["dw"]
            c0 = i * P
            if part == 0:
                cc, cw, off = c0, P, 0
            else:
                if dw <= P:
                    return
                cc, cw, off = c0 + P, dw - P, P
            if i % 2 == 0 and i >= 2:
                dr_acc(dp[:, off:off + cw], m, (i - 2) // 2, i, cc, cw, part == 1)
            elif i % 2 == 1:
                single_acc(dp[:, off:off + cw], m, i - 1, i, cc, cw, part == 1)

        def rest_chain(m, i, cx, dma=None):
            """Gram + full accum (panels <= i-1, all available when queued
            at the end of round i-1) for cols [c0+DW, c0+w), then strip
            evac.  Pumped as filler during round i, drained pre-TRSM."""
            w, rp = cx["w"], cx["rp"]
            if rp is None:
                return
            c0 = i * P
            rw = w - DW
            for kp in range(NKT // 2):
                if dma is not None:
                    dma(kp)
                for (cc, cw) in _chunks(rw, c0 + DW, DRT):
                    dr_gram(rp[:, ds(cc - c0 - DW, cw)], m, kp, c0, cc, cw)
                    yield
            pairs = list(range(i // 2))
            single_j = i - 1 if i % 2 == 1 else None
            nacc = len(pairs) + (1 if single_j is not None else 0)
            k = 0
            for pr in pairs:
                k += 1
                for (cc, cw) in _chunks(rw, c0 + DW, DRT):
                    dr_acc(rp[:, ds(cc - c0 - DW, cw)], m, pr, i, cc, cw,
                           k == nacc)
                    yield
            if single_j is not None:
                k += 1
                for (cc, cw) in _chunks(rw, c0 + DW, DRT):
                    single_acc(rp[:, ds(cc - c0 - DW, cw)], m, single_j, i,
                               cc, cw, k == nacc)
                    yield
            strip = cx["strip"]
            for (cc, cw) in _chunks(rw, 0, FT):
                nc.scalar.copy(strip[:, ds(DW - P + cc, cw)], rp[:, ds(cc, cw)])
                yield

        def evac_dstrip(m, i, cx):
            dp, w, dw = cx["dp"], cx["w"], cx["dw"]
            if w > P:
                nc.scalar.copy(cx["strip"][:, : dw - P], dp[:, ds(P, dw - P)])

        # ---------------- refine ----------------
        def rtile(pool, shape, dt, tag):
            return pool.tile(shape, dt, tag=tag, name=tag)

        def refine_gen(rnd, A):
            duo = len(A) == 2
            lo = 0 if duo else A[0][0]
            nsl = 2 if duo else 1

            def sl(t):
                return t[:, lo : lo + nsl, :]

            sblk2 = rtile(rpool, [P, 2, P], f32, "sblk2")
            c1s = rtile(rpool, [P, 2, P], f32, "c1s")
            x1 = rtile(rpool, [P, 2, P], bf16, "x1")
            x1t = rtile(rpool, [P, 2, P], bf16, "x1t")
            x1mi = rtile(rpool, [P, 2, P], f32, "x1mi")
            a2c = rtile(rpool, [P, 2, P], f32, "a2c")
            x1ms = rtile(rpool, [P, 2, P], f32, "x1ms")
            wfac = rtile(rpool, [P, 2, P], bf16, "wfac")
            sb2 = rtile(rpool, [P, 2, P], bf16, "sb2")
            swt = rtile(rpool, [P, 2, P], bf16, "swt")
            fcop = rtile(rpool, [P, 2, P], f32, "fcop")
            scr = rtile(rpool, [P, 2, P], f32, "scr")
            rrow = rtile(vpool, [1, 2, P], bf16, "rrow")
            dcol = rtile(vpool, [P, 2], f32, "dcol")
            rtmp = rtile(vpool, [P, 2], f32, "rtmp")
            for (m, i, cx) in A:
                cx["wfac"] = wfac[:, m, :]

            # stage 0: pivot copy; fused diag-extract+fix; rsqrt column
            if duo:
                nc.vector.tensor_copy(sblk2, ddiag[:, :, :P])
            else:
                nc.vector.tensor_copy(sl(sblk2), ddiag[:, lo, :P])
            if SAFE_TTR:
                nc.vector.tensor_mul(sl(scr), sl(sblk2), sl(I2f))
                nc.vector.tensor_reduce(
                    dcol[:, lo : lo + nsl], sl(scr), AX, OP.add
                )
                for (m, i, cx) in A:
                    if m == 0:
                        nc.vector.tensor_add(
                            dcol[:, 0:1], dcol[:, 0:1], pscl[:, ds(i, 1)]
                        )
                    else:
                        nc.vector.tensor_scalar(
                            out=dcol[:, 1:2], in0=dcol[:, 1:2], scalar1=1.0,
                            scalar2=float(S2), op0=OP.mult, op1=OP.add,
                        )
            else:
                for (m, i, cx) in A:
                    fix = pscl[:, ds(i, 1)] if m == 0 else float(S2)
                    nc.vector.tensor_tensor_reduce(
                        out=scr[:, m, :], in0=sblk2[:, m, :], in1=I2f[:, m, :],
                        scale=1.0, scalar=fix, op0=OP.mult, op1=OP.add,
                        accum_out=dcol[:, ds(m, 1)],
                    )
            nc.vector.reciprocal(
                dcol[:, lo : lo + nsl], dcol[:, lo : lo + nsl]
            )
            nc.scalar.sqrt(rtmp[:, lo : lo + nsl], dcol[:, lo : lo + nsl])
            yield

            # stage 1: transposes -> rrow; outer products -> q; c1
            for (m, i, cx) in A:
                nc.tensor.transpose(
                    wpsum[:1, ds(256 + m * P, P)], rtmp[:, ds(m, 1)], I128
                )
            nc.vector.tensor_copy(
                rrow[:, lo : lo + nsl, :],
                wpsum[:1, ds(256 + lo * P, nsl * P)].rearrange(
                    "p (s q) -> p s q", q=P),
            )
            for (m, i, cx) in A:
                nc.tensor.matmul(
                    wpsum[:, ds(m * P, P)], rrow[:, m, :], rrow[:, m, :],
                    start=True, stop=True, skip_group_check=True,
                )
            nc.vector.tensor_mul(
                sl(c1s), sl(sblk2),
                wpsum[:, ds(lo * P, nsl * P)].rearrange(
                    "p (s q) -> p s q", q=P),
            )
            yield

            # stage 2: triangular masks (x1t on DVE, x1/x1mi on GpSimd)
            nc.gpsimd.tensor_mul(sl(x1), sl(c1s), sl(STRIU))
            nc.vector.tensor_mul(sl(x1t), sl(c1s), sl(STRIL))
            if SAFE_GSUB:
                nc.vector.tensor_sub(sl(x1mi), sl(x1), sl(I2f))
            else:
                nc.gpsimd.tensor_sub(sl(x1mi), sl(x1), sl(I2f))
            yield

            # stage 3: x2 = X1@X1 (lo bank half), xtx = X1^T X1 (hi half)
            for (m, i, cx) in A:
                nc.tensor.matmul(
                    wpsum[:, ds(m * P, P)], x1t[:, m, :], x1[:, m, :],
                    start=True, stop=True, skip_group_check=True,
                )
            for (m, i, cx) in A:
                nc.tensor.matmul(
                    wpsum[:, ds(256 + m * P, P)], x1[:, m, :], x1[:, m, :],
                    start=True, stop=True, skip_group_check=True,
                )
            yield

            # stage 4 (all DVE): wfac = (x2 - (x1 - I - a2c)) o r
            nc.vector.tensor_mul(
                sl(a2c),
                wpsum[:, ds(256 + lo * P, nsl * P)].rearrange(
                    "p (s q) -> p s q", q=P),
                sl(STRIUH),
            )
            nc.vector.tensor_sub(sl(x1mi), sl(x1mi), sl(a2c))
            for (m, i, cx) in A:
                nc.vector.tensor_scalar_mul(
                    x1ms[:, m, :], x1mi[:, m, :], rtmp[:, ds(m, 1)]
                )
            if SAFE_STT:
                nc.vector.tensor_sub(
                    sl(scr),
                    wpsum[:, ds(lo * P, nsl * P)].rearrange(
                        "p (s q) -> p s q", q=P),
                    sl(x1mi),
                )
                for (m, i, cx) in A:
                    nc.vector.tensor_scalar_mul(
                        wfac[:, m, :], scr[:, m, :], rtmp[:, ds(m, 1)]
                    )
            else:
                for (m, i, cx) in A:
                    nc.vector.scalar_tensor_tensor(
                        out=wfac[:, m, :],
                        in0=wpsum[:, ds(m * P, P)],
                        scalar=rtmp[:, ds(m, 1)],
                        in1=x1ms[:, m, :],
                        op0=OP.mult, op1=OP.subtract,
                    )
            yield

            # stage 5 (off-path): sw matmuls; tr(F) and diag(W) accums
            for (m, i, cx) in A:
                dfix = dfix_m[:, i, :] if m == 0 else dfix_s
                nc.vector.tensor_add(sb2[:, m, :], sblk2[:, m, :], dfix)
            for (m, i, cx) in A:
                nc.tensor.matmul(
                    wpsum[:, ds(m * P, P)], sb2[:, m, :], wfac[:, m, :],
                    start=True, stop=True, skip_group_check=True,
                )
            nc.vector.tensor_copy(
                sl(swt),
                wpsum[:, ds(lo * P, nsl * P)].rearrange("p (s q) -> p s q", q=P),
            )
            if SAFE_TTR:
                nc.vector.tensor_mul(
                    sl(scr), sl(wfac),
                    wpsum[:, ds(lo * P, nsl * P)].rearrange(
                        "p (s q) -> p s q", q=P),
                )
                nc.vector.tensor_reduce(
                    dcol[:, lo : lo + nsl], sl(scr), AX, OP.add
                )
                nc.vector.tensor_add(
                    accA[:, lo : lo + nsl], accA[:, lo : lo + nsl],
                    dcol[:, lo : lo + nsl],
                )
                nc.vector.tensor_mul(sl(scr), sl(wfac), sl(I2f))
                nc.vector.tensor_reduce(
                    dstore[:, rnd, lo : lo + nsl], sl(scr), AX, OP.add
                )
            else:
                for (m, i, cx) in A:
                    nc.vector.tensor_tensor_reduce(
                        out=scr[:, m, :], in0=wfac[:, m, :],
                        in1=wpsum[:, ds(m * P, P)], scale=1.0,
                        scalar=accA[:, ds(m, 1)], op0=OP.mult, op1=OP.add,
                        accum_out=accA[:, ds(m, 1)],
                    )
                for (m, i, cx) in A:
                    nc.vector.tensor_tensor_reduce(
                        out=scr[:, m, :], in0=wfac[:, m, :], in1=I2f[:, m, :],
                        scale=1.0, scalar=0.0, op0=OP.mult, op1=OP.add,
                        accum_out=dstore[:, rnd, ds(m, 1)],
                    )
            yield

            # stage 6 (off-path): fpi matmuls; tr(F^2) accum
            for (m, i, cx) in A:
                nc.tensor.matmul(
                    wpsum[:, ds(256 + m * P, P)], wfac[:, m, :], swt[:, m, :],
                    start=True, stop=True, skip_group_check=True,
                )
            nc.vector.tensor_copy(
                sl(fcop),
                wpsum[:, ds(256 + lo * P, nsl * P)].rearrange(
                    "p (s q) -> p s q", q=P),
            )
            if SAFE_TTR:
                nc.vector.tensor_mul(sl(scr), sl(fcop), sl(fcop))
                nc.vector.tensor_reduce(
                    dcol[:, lo : lo + nsl], sl(scr), AX, OP.add
                )
                nc.vector.tensor_add(
                    accB[:, lo : lo + nsl], accB[:, lo : lo + nsl],
                    dcol[:, lo : lo + nsl],
                )
            else:
                for (m, i, cx) in A:
                    nc.vector.tensor_tensor_reduce(
                        out=scr[:, m, :],
                        in0=fcop[:, m, :],
                        in1=fcop[:, m, :],
                        scale=1.0, scalar=accB[:, ds(m, 1)],
                        op0=OP.mult, op1=OP.add,
                        accum_out=accB[:, ds(m, 1)],
                    )

        def trsm_gen(m, i, cx):
            """U_i[:, P:] = wfac^T @ strip (no diag chunk).  First chunk is
            the next panel's 128-col pivot block (fast tail unblock), then
            512-col chunks.  Chunks land in this panel's rest banks;
            overflow in wpsum.  u8 on scalar; n8 on DVE for the small
            chunk (tail parallelism) and scalar for wide chunks."""
            w = cx["w"]
            if w <= P:
                return
            c0 = i * P
            rp = cx["rp"]
            pr, slot = i // 2, i % 2
            o = 2 * pr * P
            rp_off = 0
            chunks = [(c0 + P, min(P, w - P))] + _chunks(
                max(w - 2 * P, 0), c0 + 2 * P, FT
            )
            for cidx, (cc, cw) in enumerate(chunks):
                tp = None
                if cidx == 0:
                    # next pivot block: per-matrix wpsum slot
                    tp = wpsum[:, ds(m * P, cw)]
                elif rp is not None:
                    # keep matmul outputs within one psum bank
                    aoff = rp_off
                    if aoff % FT + cw > FT:
                        aoff += FT - aoff % FT
                    if aoff + cw <= BANKW[m]:
                        tp = rp[:, ds(aoff, cw)]
                        rp_off = aoff + cw
                if tp is None:
                    tp = wpsum[:, ds(256, cw)]
                nc.tensor.matmul(
                    tp, cx["wfac"], cx["strip"][:, ds(cc - c0 - P, cw)],
                    start=True, stop=True, skip_group_check=True,
                )
                nc.scalar.copy(u8[(m, pr)][:, slot, ds(cc - o, cw)], tp)
                if cw <= P:
                    nc.vector.tensor_scalar(
                        out=n8[(m, pr)][:, slot, ds(cc - o, cw)], in0=tp,
                        scalar1=-1.0, scalar2=None, op0=OP.mult,
                    )
                else:
                    nc.scalar.mul(n8[(m, pr)][:, slot, ds(cc - o, cw)], tp, -1.0)
                yield

        # ---------------- schedule ----------------
        rm_of_round = {r: i for i, r in enumerate(RM)}
        fillers = []

        def pump_fillers(k=1, only=None):
            done = 0
            idx = 0
            while idx < len(fillers) and done < k:
                key, g = fillers[idx]
                if only is not None and key not in only:
                    idx += 1
                    continue
                try:
                    next(g)
                    done += 1
                except StopIteration:
                    fillers.pop(idx)

        # column-split input DMAs: the panel-0 diag chains only need cols
        # [0, 512), so ship those first; the rest streams under round 0.
        def dma_cols(dst, src, lo_c, hi_c):
            def dma(kp):
                for k in (2 * kp, 2 * kp + 1):
                    nc.sync.dma_start(
                        dst[:, k, ds(lo_c, hi_c - lo_c)],
                        src[:, k, ds(lo_c, hi_c - lo_c)],
                    )
            return dma

        # bootstrap
        cs = new_panel(1, 0)
        for _ in diag_chain(1, 0, cs, dma=dma_cols(bful, bb_r, 0, DW)):
            pass
        cm = new_panel(0, 0)
        fillers.append(
            [(0, 0, "d"), diag_chain(0, 0, cm, dma=dma_cols(bsel, bs_r, 0, DW))]
        )
        cur0 = {0: (0, cm)}
        fillers.append(
            [(1, 0, "r"), rest_chain(1, 0, cs, dma=dma_cols(bful, bb_r, DW, N))]
        )
        evac_dstrip(1, 0, cs)
        curA = [(1, 0, cs)]
        ref = refine_gen(0, curA)
        next(ref)   # stage 0
        pump_fillers(4)
        next(ref)   # stage 1
        pump_fillers(4)

        for r in range(NROUNDS):
            # refine stages 2..4 with filler between
            for _ in range(3):
                next(ref)
                pump_fillers(3)
            # this round's rest chains must be fully emitted before TRSM
            pump_fillers(100000, only={(m, i, "r") for (m, i, cx) in curA})

            # next round's panels: create + queue diag chains
            nxtA = []
            r1 = r + 1
            if r1 < NT:
                mi1 = rm_of_round.get(r1)
                if mi1 is not None:
                    if mi1 == 0:
                        nx = cur0[0][1]
                        pump_fillers(100000, only={(0, 0, "d")})
                    else:
                        nx = new_panel(0, mi1)
                        fillers.append([(0, mi1, "d"),
                                        diag_chain(0, mi1, nx)])
                    nxtA.append((0, mi1, nx))
                ns_ = new_panel(1, r1)
                fillers.append([(1, r1, "d"), diag_chain(1, r1, ns_)])
                nxtA.append((1, r1, ns_))

            # TRSM first chunks (next pivot block, 128 cols) + evacs
            tgens = []
            for (m, i, cx) in curA:
                g = trsm_gen(m, i, cx)
                try:
                    next(g)
                    tgens.append(g)
                except StopIteration:
                    pass
            # drain next diag chains, then the pivot-block tails
            for (m, ni, nx) in nxtA:
                pump_fillers(100000, only={(m, ni, "d")})
                diag_tail(m, ni, nx, 0)

            nref = None
            if nxtA:
                nref = refine_gen(r1, nxtA)
                next(nref)  # stage 0 (pivot row chain; runs under TRSM)

            # second TRSM chunk, then the diag-region tails
            live = []
            for g in tgens:
                try:
                    next(g)
                    live.append(g)
                except StopIteration:
                    pass
            for (m, ni, nx) in nxtA:
                diag_tail(m, ni, nx, 1)

            # remaining TRSM chunks + current refine stages 5,6 + fillers
            for g in live:
                for _ in g:
                    pump_fillers(2)
            for _ in ref:
                pump_fillers(2)

            if nxtA:
                for (m, ni, nx) in nxtA:
                    dma = None
                    if m == 0 and ni == 0:
                        dma = dma_cols(bsel, bs_r, DW, NM)
                    fillers.append([(m, ni, "r"), rest_chain(m, ni, nx, dma=dma)])
                    evac_dstrip(m, ni, nx)
                next(nref)  # stage 1
                pump_fillers(2)
                ref = nref
                curA = nxtA
        if NROUNDS < NT:
            for _ in ref:
                pump_fillers(2)
        pump_fillers(100000)

        # -------- final: batched Ln, combine, partition-sum --------
        lnall = vpool.tile([P, NT, 2], f32, tag="lnall", name="lnall")
        nc.scalar.activation(
            lnall.rearrange("p a b -> p (a b)"),
            dstore.rearrange("p a b -> p (a b)"),
            AF.Ln,
        )
        ln0 = vpool.tile([P, 1], f32, tag="ln0", name="ln0")
        nc.vector.tensor_reduce(ln0, lnall[:, :, 0:1], AXY, OP.add)
        ln1 = vpool.tile([P, 1], f32, tag="ln1", name="ln1")
        nc.vector.tensor_reduce(ln1, lnall[:, :, 1:2], AXY, OP.add)
        # acc = -2(ln0 - ln1) + 2(accA0 - accA1) - 0.5(accB0 - accB1)
        accd = vpool.tile([P, 1], f32, tag="accd", name="accd")
        t0 = vpool.tile([P, 1], f32, tag="t0", name="t0")
        nc.vector.tensor_sub(accd, ln1, ln0)
        nc.vector.tensor_scalar(
            out=accd, in0=accd, scalar1=2.0, scalar2=None, op0=OP.mult
        )
        nc.vector.tensor_sub(t0, accA[:, 0:1], accA[:, 1:2])
        nc.vector.tensor_scalar(
            out=t0, in0=t0, scalar1=2.0, scalar2=None, op0=OP.mult
        )
        nc.vector.tensor_add(accd, accd, t0)
        nc.vector.tensor_sub(t0, accB[:, 0:1], accB[:, 1:2])
        nc.vector.tensor_scalar(
            out=t0, in0=t0, scalar1=-0.5, scalar2=None, op0=OP.mult
        )
        nc.vector.tensor_add(accd, accd, t0)
        ones = vpool.tile([P, 1], f32, tag="ones", name="ones")
        nc.vector.memset(ones, 1.0)
        nc.tensor.matmul(wpsum[:1, :1], accd, ones, start=True, stop=True,
                         skip_group_check=True)
        res = vpool.tile([1, 1], f32, tag="res", name="res")
        nc.vector.tensor_scalar(
            out=res, in0=wpsum[:1, :1], scalar1=1.0, scalar2=OUT_CONST,
            op0=OP.mult, op1=OP.add,
        )
        nc.sync.dma_start(out_d[:, :], res)

    nc.finalize()
    return nc


def make_in_maps(x, B):
    bs, n = x.shape
    k = B.shape[0]
    b8 = np.zeros((N, N), dtype=ml_dtypes.float8_e4m3)
    b8[:k, :] = (B * SCALE).astype(ml_dtypes.float8_e4m3)
    rm_of_round = {r: i for i, r in enumerate(RM)}
    in_maps = []
    for c in range(bs):
        sel = np.nonzero(x[c] == 1)[0]
        ns = len(sel)
        assert ns <= NM, f"sample {c} selects {ns} > {NM} columns"
        bsel = np.zeros((N, NM), dtype=ml_dtypes.float8_e4m3)
        bsel[:, :ns] = b8[:, sel]
        pad = np.zeros((NM, 1), dtype=np.float32)
        pad[ns:] = 1.0
        in_maps.append({"bb": b8, "bs": bsel, "pad": pad})
    return in_maps


def kernel(x, B):
    """Full inputs -> full output. x: [8, 2048] int32, B: [2000, 2048] f32."""
    from concourse.bass_utils import run_bass_kernel_spmd

    bs, n = x.shape
    assert n == N and bs == 8

    if "nc" not in _CACHE:
        _CACHE["nc"] = _build()
    nc = _CACHE["nc"]

    in_maps = make_in_maps(x, B)
    res = run_bass_kernel_spmd(nc, in_maps, core_ids=list(range(bs)))
    out = np.array([r["out"][0, 0] for r in res.results], dtype=np.float32)
    return out
